# revision 4
# baseline (speedup 1.0000x reference)
"""Trainium2 Bass kernel for a 2-layer LSTM regressor (B=128, T=4096, D=64, H=512).

Strategy:
- Data-parallel over batch: 8 cores x 16 rows each; no cross-core communication.
- All per-step tensors kept "H-major" (transposed): [feature partitions, batch cols].
  The recurrent matmuls use weight tiles as the stationary operand (FWL bf16,
  ~35ns per 128x128 tile) and the 16-wide batch as the moving operand, so the
  layout is closed under the recurrence (no transposes anywhere).
- Gate order remapped to (i, f, o, g) so one sigmoid covers 3/4 of the gates.
- Biases folded into the matmuls via two constant-ones rows carrying a bf16
  hi/lo split of the fp32 bias.
- Layer-2 input projection u2 = h1 @ w_ih2.T and the output projection are
  chunk-batched over S-step blocks (moving dim S*16 = 256), layer 2 runs one
  block behind layer 1 so its tail latency hides under layer-1 matmuls.
- fp32 cell state, fp32 PSUM accumulation; only matmul operands are bf16
  (measured end-to-end error ~4e-3 rel-to-absmax).
"""
import sys
sys.path.insert(0, "/opt/trn_rl_repo")
import numpy as np
import ml_dtypes

import concourse.bass as bass
import concourse.bacc as bacc
import concourse.tile as tile
from concourse import mybir, bass_utils
from concourse.bass import ds, ts

F32 = mybir.dt.float32
BF16 = mybir.dt.bfloat16
AF = mybir.ActivationFunctionType
BF = ml_dtypes.bfloat16

B, T, D_IN = 128, 4096, 64
H, D_OUT = 512, 64
NC = 8
BC = B // NC  # 16

# new gate order (i, f, o, g) from torch order (i, f, g, o)
GATE_PERM = (0, 1, 3, 2)
ROW_PERM = np.concatenate([np.arange(512) + g * 512 for g in GATE_PERM])


# ---------------------------------------------------------------- host prep
def _prep_whh(w):
    """[2048, 512] -> [128, 16*4*128] bf16, col=(m*4+k)*128+j, m=(gate,hblock)."""
    t = w[ROW_PERM].reshape(16, 128, 4, 128)  # [m, j, k, p]
    t = t.transpose(3, 0, 2, 1)  # [p, m, k, j]
    return np.ascontiguousarray(t.reshape(128, 16 * 4 * 128)).astype(BF)


def _prep_wih1(w_ih1, b1):
    """[2048, 64] + bias -> [66, 2048] bf16 (rows 64/65 = bias hi/lo)."""
    top = w_ih1[ROW_PERM].T  # [64, 2048]
    b = b1[ROW_PERM].astype(np.float32)
    bhi = b.astype(BF).astype(np.float32)
    blo = b - bhi
    return np.concatenate([top, bhi[None], blo[None]], 0).astype(BF)


def _prep_b2(b2):
    b = b2[ROW_PERM].astype(np.float32)
    bhi = b.astype(BF).astype(np.float32)
    blo = b - bhi
    return np.stack([bhi, blo], 0).astype(BF)  # [2, 2048]


def _prep_wout(w_out):
    """[64, 512] -> [128, 4*64]: [p, k*64+d] = w_out[d, k*128+p]."""
    t = w_out.reshape(64, 4, 128).transpose(2, 1, 0)  # [p, k, d]
    return np.ascontiguousarray(t.reshape(128, 256)).astype(BF)


def _prep_bout(b_out):
    b = b_out.astype(np.float32)
    bhi = b.astype(BF).astype(np.float32)
    blo = b - bhi
    return np.stack([bhi, blo], 0).astype(BF)  # [2, 64]


def _prep_x(x_core, S):
    """[BC, T_, 64] -> (xp [66,SB], xe [NB2,66,SB], xo [NB2-1,66,SB]) bf16."""
    bc, T_, _ = x_core.shape
    NB = T_ // S
    SB = S * bc
    arr = x_core.transpose(1, 2, 0).reshape(NB, S, 64, bc).transpose(0, 2, 1, 3)
    ones = np.ones((NB, 2, S, bc), np.float32)
    xa = np.concatenate([arr, ones], 1).reshape(NB, 66, SB).astype(BF)
    return (
        np.ascontiguousarray(xa[0]),
        np.ascontiguousarray(xa[1::2]),
        np.ascontiguousarray(xa[2::2]),
    )


def _reassemble_y(ye, yo, S, bc, T_):
    NB = T_ // S
    blocks = np.empty((NB, 64, S, bc), np.float32)
    blocks[0::2] = ye.reshape(-1, 64, S, bc)
    blocks[1::2] = yo.reshape(-1, 64, S, bc)
    return blocks.transpose(3, 0, 2, 1).reshape(bc, T_, 64)


# ---------------------------------------------------------------- program
def build_program(T_=T, S=16, bc=BC, n_cores=NC):
    NB = T_ // S
    NB2 = NB // 2
    SB = S * bc
    assert NB % 2 == 0 and NB >= 4
    nc = bacc.Bacc("TRN2", target_bir_lowering=False, debug=False, num_devices=n_cores)

    d = {}
    d["w1hh"] = nc.dram_tensor("w1hh", [128, 8192], BF16, kind="ExternalInput")
    d["w1ih"] = nc.dram_tensor("w1ih", [66, 2048], BF16, kind="ExternalInput")
    d["w2hh"] = nc.dram_tensor("w2hh", [128, 8192], BF16, kind="ExternalInput")
    d["w2ih"] = nc.dram_tensor("w2ih", [128, 8192], BF16, kind="ExternalInput")
    d["b2"] = nc.dram_tensor("b2", [2, 2048], BF16, kind="ExternalInput")
    d["wout"] = nc.dram_tensor("wout", [128, 256], BF16, kind="ExternalInput")
    d["bout"] = nc.dram_tensor("bout", [2, 64], BF16, kind="ExternalInput")
    d["xp"] = nc.dram_tensor("xp", [66, SB], BF16, kind="ExternalInput")
    d["xe"] = nc.dram_tensor("xe", [NB2, 66, SB], BF16, kind="ExternalInput")
    d["xo"] = nc.dram_tensor("xo", [NB2 - 1, 66, SB], BF16, kind="ExternalInput")
    d["ye"] = nc.dram_tensor("ye", [NB2, 64, SB], F32, kind="ExternalOutput")
    d["yo"] = nc.dram_tensor("yo", [NB2, 64, SB], F32, kind="ExternalOutput")

    with tile.TileContext(nc) as tc:
        with tc.tile_pool(name="persist", bufs=1) as pp, \
             tc.tile_pool(name="work", bufs=3) as wp, \
             tc.tile_pool(name="xin", bufs=2) as xp_pool, \
             tc.tile_pool(name="psum", bufs=2, space="PSUM") as psp:

            w1hh = pp.tile([128, 8192], BF16)
            w1ih = pp.tile([66, 2048], BF16)
            w2hh = pp.tile([128, 8192], BF16)
            w2ih = pp.tile([128, 8192], BF16)
            b2sb = pp.tile([2, 2048], BF16)
            wout = pp.tile([128, 256], BF16)
            bout = pp.tile([2, 64], BF16)
            for t_, dr in [(w1hh, "w1hh"), (w1ih, "w1ih"), (w2hh, "w2hh"),
                           (w2ih, "w2ih"), (b2sb, "b2"), (wout, "wout"), (bout, "bout")]:
                nc.sync.dma_start(t_[:], d[dr].ap())

            ones = pp.tile([2, SB], BF16)
            nc.vector.memset(ones[:], 1.0)

            hist1 = [pp.tile([128, 4, (S + 1) * bc], BF16, name=f"hist1_{i}") for i in range(2)]
            hist2 = pp.tile([128, 4, (S + 1) * bc], BF16)
            c1 = pp.tile([128, 4, bc], F32)
            c2 = pp.tile([128, 4, bc], F32)
            u2sb = [pp.tile([128, 16, S, bc], F32, name=f"u2sb_{i}") for i in range(2)]
            for t_ in (hist1[0], hist1[1], hist2):
                nc.vector.memset(t_[:], 0.0)
            nc.vector.memset(c1[:], 0.0)
            nc.vector.memset(c2[:], 0.0)

            def l1_step(X, xc, s):
                """L1 step: reads X slot s (+ x col block s), writes X slot s+1."""
                g1 = psp.tile([128, 16, bc], F32, tag="g1")
                rhs_h = [X[:, k, s * bc:(s + 1) * bc] for k in range(4)]
                rhs_x = xc[:, s * bc:(s + 1) * bc]
                for m in range(16):
                    o = g1[:, m, :]
                    for k in range(4):
                        nc.tensor.matmul(o, w1hh[:, (m * 4 + k) * 128:(m * 4 + k + 1) * 128],
                                         rhs_h[k], start=(k == 0), stop=False)
                    nc.tensor.matmul(o, w1ih[:, m * 128:(m + 1) * 128], rhs_x,
                                     start=False, stop=True)
                a = wp.tile([128, 16, bc], F32, tag="a1")
                nc.scalar.activation(a[:, 0:12, :], g1[:, 0:12, :], AF.Sigmoid)
                nc.scalar.activation(a[:, 12:16, :], g1[:, 12:16, :], AF.Tanh)
                tmp = wp.tile([128, 4, bc], F32, tag="tmp1")
                nc.vector.tensor_mul(tmp[:], a[:, 0:4, :], a[:, 12:16, :])
                nc.vector.tensor_mul(c1[:], a[:, 4:8, :], c1[:])
                nc.vector.tensor_add(c1[:], c1[:], tmp[:])
                tct = wp.tile([128, 4, bc], F32, tag="tc1")
                nc.scalar.activation(tct[:], c1[:], AF.Tanh)
                nc.vector.tensor_mul(X[:, :, (s + 1) * bc:(s + 2) * bc], a[:, 8:12, :], tct[:])

            def l2_step(U, s):
                """L2 step: hist2 slot s -> slot s+1, gates = whh2 psum + u2 slice."""
                g2 = psp.tile([128, 16, bc], F32, tag="g2")
                rhs_h = [hist2[:, k, s * bc:(s + 1) * bc] for k in range(4)]
                for m in range(16):
                    o = g2[:, m, :]
                    for k in range(4):
                        nc.tensor.matmul(o, w2hh[:, (m * 4 + k) * 128:(m * 4 + k + 1) * 128],
                                         rhs_h[k], start=(k == 0), stop=(k == 3))
                gs = wp.tile([128, 16, bc], F32, tag="gs")
                nc.vector.tensor_add(gs[:], g2[:], U[:, :, s, :])
                a = wp.tile([128, 16, bc], F32, tag="a2")
                nc.scalar.activation(a[:, 0:12, :], gs[:, 0:12, :], AF.Sigmoid)
                nc.scalar.activation(a[:, 12:16, :], gs[:, 12:16, :], AF.Tanh)
                tmp = wp.tile([128, 4, bc], F32, tag="tmp2")
                nc.vector.tensor_mul(tmp[:], a[:, 0:4, :], a[:, 12:16, :])
                nc.vector.tensor_mul(c2[:], a[:, 4:8, :], c2[:])
                nc.vector.tensor_add(c2[:], c2[:], tmp[:])
                tct = wp.tile([128, 4, bc], F32, tag="tc2")
                nc.scalar.activation(tct[:], c2[:], AF.Tanh)
                nc.vector.tensor_mul(hist2[:, :, (s + 1) * bc:(s + 2) * bc], a[:, 8:12, :], tct[:])

            def u2chunk(Xr, U):
                """u2 = w_ih2-proj of block's h1 (slots 1..S of Xr) + b2 -> U."""
                for m in range(16):
                    up = psp.tile([128, S, bc], F32, tag="u2")
                    for k in range(4):
                        nc.tensor.matmul(up[:], w2ih[:, (m * 4 + k) * 128:(m * 4 + k + 1) * 128],
                                         Xr[:, k, bc:], start=(k == 0), stop=False)
                    nc.tensor.matmul(up[:], b2sb[:, m * 128:(m + 1) * 128], ones[:],
                                     start=False, stop=True)
                    nc.scalar.copy(U[:, m], up[:])

            def ychunk(y_ap):
                yp = psp.tile([64, SB], F32, tag="y")
                for k in range(4):
                    nc.tensor.matmul(yp[:], wout[:, k * 64:(k + 1) * 64],
                                     hist2[:, k, bc:], start=(k == 0), stop=False)
                nc.tensor.matmul(yp[:], bout[:], ones[:], start=False, stop=True)
                ysb = wp.tile([64, SB], F32, tag="ysb")
                nc.scalar.copy(ysb[:], yp[:])
                nc.sync.dma_start(y_ap, ysb[:])

            def carry(Xw, Xr):
                nc.vector.tensor_copy(Xw[:, :, 0:bc], Xr[:, :, S * bc:(S + 1) * bc])

            def half(par, x_ap, y_ap, has_l1=True):
                """One half: L1 on block h+1 (parity buffers), L2 on block h, y(h)."""
                Xw = hist1[par]       # written by L1 this half
                Xr = hist1[1 - par]   # h1 of block h (read by u2chunk)
                U = u2sb[par]
                if has_l1:
                    carry(Xw, Xr)
                nc.vector.tensor_copy(hist2[:, :, 0:bc], hist2[:, :, S * bc:(S + 1) * bc])
                if has_l1:
                    xc = xp_pool.tile([66, SB], BF16, tag="xc")
                    nc.sync.dma_start(xc[:], x_ap)
                u2chunk(Xr, U)
                for s in range(S):
                    if has_l1:
                        l1_step(Xw, xc, s)
                    l2_step(U, s)
                ychunk(y_ap)

            # prologue: L1 on block 0 into hist1[1]
            xc0 = xp_pool.tile([66, SB], BF16, tag="xc")
            nc.sync.dma_start(xc0[:], d["xp"].ap())
            for s in range(S):
                l1_step(hist1[1], xc0, s)

            if NB2 >= 2:
                with tc.For_i(0, NB2 - 1, 1) as j:
                    half(0, d["xe"].ap()[ds(j, 1)], d["ye"].ap()[ds(j, 1)])
                    half(1, d["xo"].ap()[ds(j, 1)], d["yo"].ap()[ds(j, 1)])
            # epilogue: halves NB-2 and NB-1
            half(0, d["xe"].ap()[NB2 - 1], d["ye"].ap()[NB2 - 1])
            half(1, None, d["yo"].ap()[NB2 - 1], has_l1=False)

    nc.compile()
    return nc


_CACHE = {}


def _get_program(T_, S):
    key = (T_, S)
    if key not in _CACHE:
        _CACHE[key] = build_program(T_=T_, S=S)
    return _CACHE[key]


def kernel(x, w_ih1, w_hh1, b_ih1, b_hh1, w_ih2, w_hh2, b_ih2, b_hh2, w_out, b_out,
           _T=None, _S=16):
    x = np.asarray(x, dtype=np.float32)
    T_ = x.shape[1] if _T is None else _T
    nc = _get_program(T_, _S)

    shared = {
        "w1hh": _prep_whh(np.asarray(w_hh1)),
        "w1ih": _prep_wih1(np.asarray(w_ih1), np.asarray(b_ih1) + np.asarray(b_hh1)),
        "w2hh": _prep_whh(np.asarray(w_hh2)),
        "w2ih": _prep_whh(np.asarray(w_ih2)),
        "b2": _prep_b2(np.asarray(b_ih2) + np.asarray(b_hh2)),
        "wout": _prep_wout(np.asarray(w_out)),
        "bout": _prep_bout(np.asarray(b_out)),
    }
    in_maps = []
    for c in range(NC):
        xp_, xe_, xo_ = _prep_x(x[c * BC:(c + 1) * BC], _S)
        in_maps.append({**shared, "xp": xp_, "xe": xe_, "xo": xo_})

    res = bass_utils.run_bass_kernel_spmd(nc, in_maps, core_ids=list(range(NC)))
    y = np.empty((B, T_, D_OUT), np.float32)
    for c in range(NC):
        y[c * BC:(c + 1) * BC] = _reassemble_y(
            res.results[c]["ye"], res.results[c]["yo"], _S, BC, T_)
    return y


# revision 5
# speedup vs baseline: 1.0690x; 1.0690x over previous
"""Trainium2 Bass kernel for a 2-layer LSTM regressor (B=128, T=4096, D=64, H=512).

Strategy:
- Data-parallel over batch: 8 cores x 16 rows each; no cross-core communication.
- All per-step tensors kept "H-major" (transposed): [feature partitions, batch cols].
  The recurrent matmuls use weight tiles as the stationary operand (FWL bf16,
  ~35ns per 128x128 tile) and the 16-wide batch as the moving operand, so the
  layout is closed under the recurrence (no transposes anywhere).
- Gate order remapped to (i, f, o, g) so one sigmoid covers 3/4 of the gates.
- Biases folded into the matmuls via two constant-ones rows carrying a bf16
  hi/lo split of the fp32 bias.
- Layer-2 input projection u2 = h1 @ w_ih2.T and the output projection are
  chunk-batched over S-step blocks (moving dim S*16 = 256), layer 2 runs one
  block behind layer 1 so its tail latency hides under layer-1 matmuls.
- fp32 cell state, fp32 PSUM accumulation; only matmul operands are bf16
  (measured end-to-end error ~4e-3 rel-to-absmax).
"""
import sys
sys.path.insert(0, "/opt/trn_rl_repo")
import numpy as np
import ml_dtypes

import concourse.bass as bass
import concourse.bacc as bacc
import concourse.tile as tile
from concourse import mybir, bass_utils
from concourse.bass import ds, ts

F32 = mybir.dt.float32
BF16 = mybir.dt.bfloat16
AF = mybir.ActivationFunctionType
BF = ml_dtypes.bfloat16

B, T, D_IN = 128, 4096, 64
H, D_OUT = 512, 64
NC = 8
BC = B // NC  # 16

# new gate order (i, f, o, g) from torch order (i, f, g, o)
GATE_PERM = (0, 1, 3, 2)
ROW_PERM = np.concatenate([np.arange(512) + g * 512 for g in GATE_PERM])


# ---------------------------------------------------------------- host prep
def _prep_whh(w):
    """[2048, 512] -> [128, 16*4*128] bf16, col=(m*4+k)*128+j, m=(gate,hblock)."""
    t = w[ROW_PERM].reshape(16, 128, 4, 128)  # [m, j, k, p]
    t = t.transpose(3, 0, 2, 1)  # [p, m, k, j]
    return np.ascontiguousarray(t.reshape(128, 16 * 4 * 128)).astype(BF)


def _prep_wih1(w_ih1, b1):
    """[2048, 64] + bias -> [66, 2048] bf16 (rows 64/65 = bias hi/lo)."""
    top = w_ih1[ROW_PERM].T  # [64, 2048]
    b = b1[ROW_PERM].astype(np.float32)
    bhi = b.astype(BF).astype(np.float32)
    blo = b - bhi
    return np.concatenate([top, bhi[None], blo[None]], 0).astype(BF)


def _prep_b2(b2):
    b = b2[ROW_PERM].astype(np.float32)
    bhi = b.astype(BF).astype(np.float32)
    blo = b - bhi
    return np.stack([bhi, blo], 0).astype(BF)  # [2, 2048]


def _prep_wout(w_out):
    """[64, 512] -> [128, 4*64]: [p, k*64+d] = w_out[d, k*128+p]."""
    t = w_out.reshape(64, 4, 128).transpose(2, 1, 0)  # [p, k, d]
    return np.ascontiguousarray(t.reshape(128, 256)).astype(BF)


def _prep_bout(b_out):
    b = b_out.astype(np.float32)
    bhi = b.astype(BF).astype(np.float32)
    blo = b - bhi
    return np.stack([bhi, blo], 0).astype(BF)  # [2, 64]


def _prep_x(x_core, S):
    """[BC, T_, 64] -> (xp [66,SB], xe [NB2,66,SB], xo [NB2-1,66,SB]) bf16."""
    bc, T_, _ = x_core.shape
    NB = T_ // S
    SB = S * bc
    arr = x_core.transpose(1, 2, 0).reshape(NB, S, 64, bc).transpose(0, 2, 1, 3)
    ones = np.ones((NB, 2, S, bc), np.float32)
    xa = np.concatenate([arr, ones], 1).reshape(NB, 66, SB).astype(BF)
    return (
        np.ascontiguousarray(xa[0]),
        np.ascontiguousarray(xa[1::2]),
        np.ascontiguousarray(xa[2::2]),
    )


def _reassemble_y(ye, yo, S, bc, T_):
    NB = T_ // S
    blocks = np.empty((NB, 64, S, bc), np.float32)
    blocks[0::2] = ye.reshape(-1, 64, S, bc)
    blocks[1::2] = yo.reshape(-1, 64, S, bc)
    return blocks.transpose(3, 0, 2, 1).reshape(bc, T_, 64)


# ---------------------------------------------------------------- program
def build_program(T_=T, S=16, bc=BC, n_cores=NC):
    NB = T_ // S
    NB2 = NB // 2
    SB = S * bc
    assert NB % 2 == 0 and NB >= 4
    nc = bacc.Bacc("TRN2", target_bir_lowering=False, debug=False, num_devices=n_cores)

    d = {}
    d["w1hh"] = nc.dram_tensor("w1hh", [128, 8192], BF16, kind="ExternalInput")
    d["w1ih"] = nc.dram_tensor("w1ih", [66, 2048], BF16, kind="ExternalInput")
    d["w2hh"] = nc.dram_tensor("w2hh", [128, 8192], BF16, kind="ExternalInput")
    d["w2ih"] = nc.dram_tensor("w2ih", [128, 8192], BF16, kind="ExternalInput")
    d["b2"] = nc.dram_tensor("b2", [2, 2048], BF16, kind="ExternalInput")
    d["wout"] = nc.dram_tensor("wout", [128, 256], BF16, kind="ExternalInput")
    d["bout"] = nc.dram_tensor("bout", [2, 64], BF16, kind="ExternalInput")
    d["xp"] = nc.dram_tensor("xp", [66, SB], BF16, kind="ExternalInput")
    d["xe"] = nc.dram_tensor("xe", [NB2, 66, SB], BF16, kind="ExternalInput")
    d["xo"] = nc.dram_tensor("xo", [NB2 - 1, 66, SB], BF16, kind="ExternalInput")
    d["ye"] = nc.dram_tensor("ye", [NB2, 64, SB], F32, kind="ExternalOutput")
    d["yo"] = nc.dram_tensor("yo", [NB2, 64, SB], F32, kind="ExternalOutput")

    with tile.TileContext(nc) as tc:
        with tc.tile_pool(name="persist", bufs=1) as pp, \
             tc.tile_pool(name="work", bufs=3) as wp, \
             tc.tile_pool(name="xin", bufs=2) as xp_pool, \
             tc.tile_pool(name="psum", bufs=2, space="PSUM") as psp:

            w1hh = pp.tile([128, 8192], BF16)
            w1ih = pp.tile([66, 2048], BF16)
            w2hh = pp.tile([128, 8192], BF16)
            w2ih = pp.tile([128, 8192], BF16)
            b2sb = pp.tile([2, 2048], BF16)
            wout = pp.tile([128, 256], BF16)
            bout = pp.tile([2, 64], BF16)
            for t_, dr in [(w1hh, "w1hh"), (w1ih, "w1ih"), (w2hh, "w2hh"),
                           (w2ih, "w2ih"), (b2sb, "b2"), (wout, "wout"), (bout, "bout")]:
                nc.sync.dma_start(t_[:], d[dr].ap())

            ones = pp.tile([2, SB], BF16)
            nc.vector.memset(ones[:], 1.0)

            hist1 = [pp.tile([128, 4, (S + 1) * bc], BF16, name=f"hist1_{i}") for i in range(2)]
            hist2 = pp.tile([128, 4, (S + 1) * bc], BF16)
            c1 = pp.tile([128, 4, bc], F32)
            c2 = pp.tile([128, 4, bc], F32)
            u2sb = [pp.tile([128, 16, S, bc], F32, name=f"u2sb_{i}") for i in range(2)]
            for t_ in (hist1[0], hist1[1], hist2):
                nc.vector.memset(t_[:], 0.0)
            nc.vector.memset(c1[:], 0.0)
            nc.vector.memset(c2[:], 0.0)

            def l1_step(X, xc, s):
                """L1 step: reads X slot s (+ x col block s), writes X slot s+1."""
                g1 = psp.tile([128, 16, bc], F32, tag="g1")
                rhs_h = [X[:, k, s * bc:(s + 1) * bc] for k in range(4)]
                rhs_x = xc[:, s * bc:(s + 1) * bc]
                for m in range(16):
                    o = g1[:, m, :]
                    for k in range(4):
                        nc.tensor.matmul(o, w1hh[:, (m * 4 + k) * 128:(m * 4 + k + 1) * 128],
                                         rhs_h[k], start=(k == 0), stop=False)
                    nc.tensor.matmul(o, w1ih[:, m * 128:(m + 1) * 128], rhs_x,
                                     start=False, stop=True)
                a = wp.tile([128, 16, bc], F32, tag="a1")
                nc.scalar.activation(a[:, 0:12, :], g1[:, 0:12, :], AF.Sigmoid)
                nc.scalar.activation(a[:, 12:16, :], g1[:, 12:16, :], AF.Tanh)
                tmp = wp.tile([128, 4, bc], F32, tag="tmp1")
                nc.vector.tensor_mul(tmp[:], a[:, 0:4, :], a[:, 12:16, :])
                nc.vector.tensor_mul(c1[:], a[:, 4:8, :], c1[:])
                nc.vector.tensor_add(c1[:], c1[:], tmp[:])
                tct = wp.tile([128, 4, bc], F32, tag="tc1")
                nc.scalar.activation(tct[:], c1[:], AF.Tanh)
                nc.vector.tensor_mul(X[:, :, (s + 1) * bc:(s + 2) * bc], a[:, 8:12, :], tct[:])

            def l2_step(U, s):
                """L2 step: hist2 slot s -> slot s+1, gates = whh2 psum + u2 slice."""
                g2 = psp.tile([128, 16, bc], F32, tag="g2")
                rhs_h = [hist2[:, k, s * bc:(s + 1) * bc] for k in range(4)]
                for m in range(16):
                    o = g2[:, m, :]
                    for k in range(4):
                        nc.tensor.matmul(o, w2hh[:, (m * 4 + k) * 128:(m * 4 + k + 1) * 128],
                                         rhs_h[k], start=(k == 0), stop=(k == 3))
                gs = wp.tile([128, 16, bc], F32, tag="gs")
                nc.vector.tensor_add(gs[:], g2[:], U[:, :, s, :])
                a = wp.tile([128, 16, bc], F32, tag="a2")
                nc.scalar.activation(a[:, 0:12, :], gs[:, 0:12, :], AF.Sigmoid)
                nc.scalar.activation(a[:, 12:16, :], gs[:, 12:16, :], AF.Tanh)
                tmp = wp.tile([128, 4, bc], F32, tag="tmp2")
                nc.vector.tensor_mul(tmp[:], a[:, 0:4, :], a[:, 12:16, :])
                nc.vector.tensor_mul(c2[:], a[:, 4:8, :], c2[:])
                nc.vector.tensor_add(c2[:], c2[:], tmp[:])
                tct = wp.tile([128, 4, bc], F32, tag="tc2")
                nc.scalar.activation(tct[:], c2[:], AF.Tanh)
                nc.vector.tensor_mul(hist2[:, :, (s + 1) * bc:(s + 2) * bc], a[:, 8:12, :], tct[:])

            def u2chunk(Xr, U):
                """u2 = w_ih2-proj of block's h1 (slots 1..S of Xr) + b2 -> U."""
                for m in range(16):
                    up = psp.tile([128, S, bc], F32, tag="u2")
                    for k in range(4):
                        nc.tensor.matmul(up[:], w2ih[:, (m * 4 + k) * 128:(m * 4 + k + 1) * 128],
                                         Xr[:, k, bc:], start=(k == 0), stop=False)
                    nc.tensor.matmul(up[:], b2sb[:, m * 128:(m + 1) * 128], ones[:],
                                     start=False, stop=True)
                    nc.scalar.copy(U[:, m], up[:])

            def ychunk(y_ap):
                yp = psp.tile([64, SB], F32, tag="y")
                for k in range(4):
                    nc.tensor.matmul(yp[:], wout[:, k * 64:(k + 1) * 64],
                                     hist2[:, k, bc:], start=(k == 0), stop=False)
                nc.tensor.matmul(yp[:], bout[:], ones[:], start=False, stop=True)
                ysb = wp.tile([64, SB], F32, tag="ysb")
                nc.scalar.copy(ysb[:], yp[:])
                nc.sync.dma_start(y_ap, ysb[:])

            def carry(Xw, Xr):
                nc.vector.tensor_copy(Xw[:, :, 0:bc], Xr[:, :, S * bc:(S + 1) * bc])

            def half(par, x_ap, y_ap, has_l1=True):
                """One half: L1 on block h+1 (parity buffers), L2 on block h, y(h)."""
                Xw = hist1[par]       # written by L1 this half
                Xr = hist1[1 - par]   # h1 of block h (read by u2chunk)
                U = u2sb[par]
                if has_l1:
                    carry(Xw, Xr)
                nc.vector.tensor_copy(hist2[:, :, 0:bc], hist2[:, :, S * bc:(S + 1) * bc])
                if has_l1:
                    xc = xp_pool.tile([66, SB], BF16, tag="xc")
                    nc.sync.dma_start(xc[:], x_ap)
                u2chunk(Xr, U)
                for s in range(S):
                    l2_step(U, s)
                    if has_l1:
                        l1_step(Xw, xc, s)
                ychunk(y_ap)

            # prologue: L1 on block 0 into hist1[1]
            xc0 = xp_pool.tile([66, SB], BF16, tag="xc")
            nc.sync.dma_start(xc0[:], d["xp"].ap())
            for s in range(S):
                l1_step(hist1[1], xc0, s)

            if NB2 >= 2:
                with tc.For_i(0, NB2 - 1, 1, hint_engines=(mybir.EngineType.PE, mybir.EngineType.Activation, mybir.EngineType.DVE, mybir.EngineType.SP)) as j:
                    half(0, d["xe"].ap()[ds(j, 1)], d["ye"].ap()[ds(j, 1)])
                    half(1, d["xo"].ap()[ds(j, 1)], d["yo"].ap()[ds(j, 1)])
            # epilogue: halves NB-2 and NB-1
            half(0, d["xe"].ap()[NB2 - 1], d["ye"].ap()[NB2 - 1])
            half(1, None, d["yo"].ap()[NB2 - 1], has_l1=False)

    nc.compile()
    return nc


_CACHE = {}


def _get_program(T_, S):
    key = (T_, S)
    if key not in _CACHE:
        _CACHE[key] = build_program(T_=T_, S=S)
    return _CACHE[key]


def kernel(x, w_ih1, w_hh1, b_ih1, b_hh1, w_ih2, w_hh2, b_ih2, b_hh2, w_out, b_out,
           _T=None, _S=16):
    x = np.asarray(x, dtype=np.float32)
    T_ = x.shape[1] if _T is None else _T
    nc = _get_program(T_, _S)

    shared = {
        "w1hh": _prep_whh(np.asarray(w_hh1)),
        "w1ih": _prep_wih1(np.asarray(w_ih1), np.asarray(b_ih1) + np.asarray(b_hh1)),
        "w2hh": _prep_whh(np.asarray(w_hh2)),
        "w2ih": _prep_whh(np.asarray(w_ih2)),
        "b2": _prep_b2(np.asarray(b_ih2) + np.asarray(b_hh2)),
        "wout": _prep_wout(np.asarray(w_out)),
        "bout": _prep_bout(np.asarray(b_out)),
    }
    in_maps = []
    for c in range(NC):
        xp_, xe_, xo_ = _prep_x(x[c * BC:(c + 1) * BC], _S)
        in_maps.append({**shared, "xp": xp_, "xe": xe_, "xo": xo_})

    res = bass_utils.run_bass_kernel_spmd(nc, in_maps, core_ids=list(range(NC)))
    y = np.empty((B, T_, D_OUT), np.float32)
    for c in range(NC):
        y[c * BC:(c + 1) * BC] = _reassemble_y(
            res.results[c]["ye"], res.results[c]["yo"], _S, BC, T_)
    return y


# revision 6
# speedup vs baseline: 1.0696x; 1.0005x over previous
"""Trainium2 Bass kernel for a 2-layer LSTM regressor (B=128, T=4096, D=64, H=512).

Strategy:
- Data-parallel over batch: 8 cores x 16 rows each; no cross-core communication.
- All per-step tensors kept "H-major" (transposed): [feature partitions, batch cols].
  The recurrent matmuls use weight tiles as the stationary operand (FWL bf16,
  ~35ns per 128x128 tile) and the 16-wide batch as the moving operand, so the
  layout is closed under the recurrence (no transposes anywhere).
- Gate order remapped to (i, f, o, g) so one sigmoid covers 3/4 of the gates.
- Biases folded into the matmuls via two constant-ones rows carrying a bf16
  hi/lo split of the fp32 bias.
- Layer-2 input projection u2 = h1 @ w_ih2.T and the output projection are
  chunk-batched over S-step blocks (moving dim S*16 = 256), layer 2 runs one
  block behind layer 1 so its tail latency hides under layer-1 matmuls.
- fp32 cell state, fp32 PSUM accumulation; only matmul operands are bf16
  (measured end-to-end error ~4e-3 rel-to-absmax).
"""
import sys
sys.path.insert(0, "/opt/trn_rl_repo")
import numpy as np
import ml_dtypes

import concourse.bass as bass
import concourse.bacc as bacc
import concourse.tile as tile
from concourse import mybir, bass_utils
from concourse.bass import ds, ts

F32 = mybir.dt.float32
BF16 = mybir.dt.bfloat16
AF = mybir.ActivationFunctionType
BF = ml_dtypes.bfloat16

B, T, D_IN = 128, 4096, 64
H, D_OUT = 512, 64
NC = 8
BC = B // NC  # 16

# new gate order (i, f, o, g) from torch order (i, f, g, o)
GATE_PERM = (0, 1, 3, 2)
ROW_PERM = np.concatenate([np.arange(512) + g * 512 for g in GATE_PERM])


# ---------------------------------------------------------------- host prep
def _prep_whh(w):
    """[2048, 512] -> [128, 16*4*128] bf16, col=(m*4+k)*128+j, m=(gate,hblock)."""
    t = w[ROW_PERM].reshape(16, 128, 4, 128)  # [m, j, k, p]
    t = t.transpose(3, 0, 2, 1)  # [p, m, k, j]
    return np.ascontiguousarray(t.reshape(128, 16 * 4 * 128)).astype(BF)


def _prep_wih1(w_ih1, b1):
    """[2048, 64] + bias -> [66, 2048] bf16 (rows 64/65 = bias hi/lo)."""
    top = w_ih1[ROW_PERM].T  # [64, 2048]
    b = b1[ROW_PERM].astype(np.float32)
    bhi = b.astype(BF).astype(np.float32)
    blo = b - bhi
    return np.concatenate([top, bhi[None], blo[None]], 0).astype(BF)


def _prep_b2(b2):
    b = b2[ROW_PERM].astype(np.float32)
    bhi = b.astype(BF).astype(np.float32)
    blo = b - bhi
    return np.stack([bhi, blo], 0).astype(BF)  # [2, 2048]


def _prep_wout(w_out):
    """[64, 512] -> [128, 4*64]: [p, k*64+d] = w_out[d, k*128+p]."""
    t = w_out.reshape(64, 4, 128).transpose(2, 1, 0)  # [p, k, d]
    return np.ascontiguousarray(t.reshape(128, 256)).astype(BF)


def _prep_bout(b_out):
    b = b_out.astype(np.float32)
    bhi = b.astype(BF).astype(np.float32)
    blo = b - bhi
    return np.stack([bhi, blo], 0).astype(BF)  # [2, 64]


def _prep_x(x_core, S):
    """[BC, T_, 64] -> (xp [66,SB], xe [NB2,66,SB], xo [NB2-1,66,SB]) bf16."""
    bc, T_, _ = x_core.shape
    NB = T_ // S
    SB = S * bc
    arr = x_core.transpose(1, 2, 0).reshape(NB, S, 64, bc).transpose(0, 2, 1, 3)
    ones = np.ones((NB, 2, S, bc), np.float32)
    xa = np.concatenate([arr, ones], 1).reshape(NB, 66, SB).astype(BF)
    return (
        np.ascontiguousarray(xa[0]),
        np.ascontiguousarray(xa[1::2]),
        np.ascontiguousarray(xa[2::2]),
    )


def _reassemble_y(ye, yo, S, bc, T_):
    NB = T_ // S
    blocks = np.empty((NB, 64, S, bc), np.float32)
    blocks[0::2] = ye.reshape(-1, 64, S, bc)
    blocks[1::2] = yo.reshape(-1, 64, S, bc)
    return blocks.transpose(3, 0, 2, 1).reshape(bc, T_, 64)


# ---------------------------------------------------------------- program
def build_program(T_=T, S=16, bc=BC, n_cores=NC):
    NB = T_ // S
    NB2 = NB // 2
    SB = S * bc
    assert NB % 2 == 0 and NB >= 4
    nc = bacc.Bacc("TRN2", target_bir_lowering=False, debug=False, num_devices=n_cores)

    d = {}
    d["w1hh"] = nc.dram_tensor("w1hh", [128, 8192], BF16, kind="ExternalInput")
    d["w1ih"] = nc.dram_tensor("w1ih", [66, 2048], BF16, kind="ExternalInput")
    d["w2hh"] = nc.dram_tensor("w2hh", [128, 8192], BF16, kind="ExternalInput")
    d["w2ih"] = nc.dram_tensor("w2ih", [128, 8192], BF16, kind="ExternalInput")
    d["b2"] = nc.dram_tensor("b2", [2, 2048], BF16, kind="ExternalInput")
    d["wout"] = nc.dram_tensor("wout", [128, 256], BF16, kind="ExternalInput")
    d["bout"] = nc.dram_tensor("bout", [2, 64], BF16, kind="ExternalInput")
    d["xp"] = nc.dram_tensor("xp", [66, SB], BF16, kind="ExternalInput")
    d["xe"] = nc.dram_tensor("xe", [NB2, 66, SB], BF16, kind="ExternalInput")
    d["xo"] = nc.dram_tensor("xo", [NB2 - 1, 66, SB], BF16, kind="ExternalInput")
    d["ye"] = nc.dram_tensor("ye", [NB2, 64, SB], F32, kind="ExternalOutput")
    d["yo"] = nc.dram_tensor("yo", [NB2, 64, SB], F32, kind="ExternalOutput")

    with tile.TileContext(nc) as tc:
        with tc.tile_pool(name="persist", bufs=1) as pp, \
             tc.tile_pool(name="work", bufs=3) as wp, \
             tc.tile_pool(name="xin", bufs=2) as xp_pool, \
             tc.tile_pool(name="psum", bufs=2, space="PSUM") as psp:

            w1hh = pp.tile([128, 8192], BF16)
            w1ih = pp.tile([66, 2048], BF16)
            w2hh = pp.tile([128, 8192], BF16)
            w2ih = pp.tile([128, 8192], BF16)
            b2sb = pp.tile([2, 2048], BF16)
            wout = pp.tile([128, 256], BF16)
            bout = pp.tile([2, 64], BF16)
            for t_, dr in [(w1hh, "w1hh"), (w1ih, "w1ih"), (w2hh, "w2hh"),
                           (w2ih, "w2ih"), (b2sb, "b2"), (wout, "wout"), (bout, "bout")]:
                nc.sync.dma_start(t_[:], d[dr].ap())

            ones = pp.tile([2, SB], BF16)
            nc.vector.memset(ones[:], 1.0)

            hist1 = [pp.tile([128, 4, (S + 1) * bc], BF16, name=f"hist1_{i}") for i in range(2)]
            hist2 = pp.tile([128, 4, (S + 1) * bc], BF16)
            c1 = pp.tile([128, 4, bc], F32)
            c2 = pp.tile([128, 4, bc], F32)
            u2sb = [pp.tile([128, 16, S, bc], F32, name=f"u2sb_{i}") for i in range(2)]
            for t_ in (hist1[0], hist1[1], hist2):
                nc.vector.memset(t_[:], 0.0)
            nc.vector.memset(c1[:], 0.0)
            nc.vector.memset(c2[:], 0.0)

            def l1_step(X, xc, s):
                """L1 step: reads X slot s (+ x col block s), writes X slot s+1."""
                g1 = psp.tile([128, 16, bc], F32, tag="g1")
                rhs_h = [X[:, k, s * bc:(s + 1) * bc] for k in range(4)]
                rhs_x = xc[:, s * bc:(s + 1) * bc]
                for m in range(16):
                    o = g1[:, m, :]
                    for k in range(4):
                        nc.tensor.matmul(o, w1hh[:, (m * 4 + k) * 128:(m * 4 + k + 1) * 128],
                                         rhs_h[k], start=(k == 0), stop=False)
                    nc.tensor.matmul(o, w1ih[:, m * 128:(m + 1) * 128], rhs_x,
                                     start=False, stop=True)
                a = wp.tile([128, 16, bc], F32, tag="a1")
                nc.scalar.activation(a[:, 0:12, :], g1[:, 0:12, :], AF.Sigmoid)
                nc.scalar.activation(a[:, 12:16, :], g1[:, 12:16, :], AF.Tanh)
                tmp = wp.tile([128, 4, bc], F32, tag="tmp1")
                nc.vector.tensor_mul(tmp[:], a[:, 0:4, :], a[:, 12:16, :])
                nc.vector.tensor_mul(c1[:], a[:, 4:8, :], c1[:])
                nc.vector.tensor_add(c1[:], c1[:], tmp[:])
                tct = wp.tile([128, 4, bc], F32, tag="tc1")
                nc.scalar.activation(tct[:], c1[:], AF.Tanh)
                nc.vector.tensor_mul(X[:, :, (s + 1) * bc:(s + 2) * bc], a[:, 8:12, :], tct[:])

            def l2_step(U, s):
                """L2 step: hist2 slot s -> slot s+1, gates = whh2 psum + u2 slice."""
                g2 = psp.tile([128, 16, bc], F32, tag="g2")
                rhs_h = [hist2[:, k, s * bc:(s + 1) * bc] for k in range(4)]
                for m in range(16):
                    o = g2[:, m, :]
                    for k in range(4):
                        nc.tensor.matmul(o, w2hh[:, (m * 4 + k) * 128:(m * 4 + k + 1) * 128],
                                         rhs_h[k], start=(k == 0), stop=(k == 3))
                gs = wp.tile([128, 16, bc], F32, tag="gs")
                nc.vector.tensor_add(gs[:], g2[:], U[:, :, s, :])
                a = wp.tile([128, 16, bc], F32, tag="a2")
                nc.scalar.activation(a[:, 0:12, :], gs[:, 0:12, :], AF.Sigmoid)
                nc.scalar.activation(a[:, 12:16, :], gs[:, 12:16, :], AF.Tanh)
                tmp = wp.tile([128, 4, bc], F32, tag="tmp2")
                nc.vector.tensor_mul(tmp[:], a[:, 0:4, :], a[:, 12:16, :])
                nc.vector.tensor_mul(c2[:], a[:, 4:8, :], c2[:])
                nc.vector.tensor_add(c2[:], c2[:], tmp[:])
                tct = wp.tile([128, 4, bc], F32, tag="tc2")
                nc.scalar.activation(tct[:], c2[:], AF.Tanh)
                nc.vector.tensor_mul(hist2[:, :, (s + 1) * bc:(s + 2) * bc], a[:, 8:12, :], tct[:])

            def u2chunk(Xr, U):
                """u2 = w_ih2-proj of block's h1 (slots 1..S of Xr) + b2 -> U."""
                for m in range(16):
                    up = psp.tile([128, S, bc], F32, tag="u2")
                    for k in range(4):
                        nc.tensor.matmul(up[:], w2ih[:, (m * 4 + k) * 128:(m * 4 + k + 1) * 128],
                                         Xr[:, k, bc:], start=(k == 0), stop=False)
                    nc.tensor.matmul(up[:], b2sb[:, m * 128:(m + 1) * 128], ones[:],
                                     start=False, stop=True)
                    nc.scalar.copy(U[:, m], up[:])

            def ychunk(y_ap):
                yp = psp.tile([64, SB], F32, tag="y")
                for k in range(4):
                    nc.tensor.matmul(yp[:], wout[:, k * 64:(k + 1) * 64],
                                     hist2[:, k, bc:], start=(k == 0), stop=False)
                nc.tensor.matmul(yp[:], bout[:], ones[:], start=False, stop=True)
                ysb = wp.tile([64, SB], F32, tag="ysb")
                nc.scalar.copy(ysb[:], yp[:])
                nc.sync.dma_start(y_ap, ysb[:])

            def carry(Xw, Xr):
                nc.vector.tensor_copy(Xw[:, :, 0:bc], Xr[:, :, S * bc:(S + 1) * bc])

            def half(par, x_ap, y_ap, has_l1=True):
                """One half: L1 on block h+1 (parity buffers), L2 on block h, y(h)."""
                Xw = hist1[par]       # written by L1 this half
                Xr = hist1[1 - par]   # h1 of block h (read by u2chunk)
                U = u2sb[par]
                if has_l1:
                    carry(Xw, Xr)
                nc.vector.tensor_copy(hist2[:, :, 0:bc], hist2[:, :, S * bc:(S + 1) * bc])
                if has_l1:
                    xc = xp_pool.tile([66, SB], BF16, tag="xc")
                    nc.sync.dma_start(xc[:], x_ap)
                u2chunk(Xr, U)
                for s in range(S):
                    l2_step(U, s)
                    if has_l1:
                        l1_step(Xw, xc, s)
                ychunk(y_ap)

            # prologue: L1 on block 0 into hist1[1]
            xc0 = xp_pool.tile([66, SB], BF16, tag="xc")
            nc.sync.dma_start(xc0[:], d["xp"].ap())
            for s in range(S):
                l1_step(hist1[1], xc0, s)

            if NB2 >= 2:
                with tc.For_i(0, NB2 - 1, 1, hint_engines=(mybir.EngineType.PE, mybir.EngineType.Activation, mybir.EngineType.DVE, mybir.EngineType.SP)) as j:
                    half(0, d["xe"].ap()[ds(j, 1)], d["ye"].ap()[ds(j, 1)])
                    half(1, d["xo"].ap()[ds(j, 1)], d["yo"].ap()[ds(j, 1)])
            # epilogue: halves NB-2 and NB-1
            half(0, d["xe"].ap()[NB2 - 1], d["ye"].ap()[NB2 - 1])
            half(1, None, d["yo"].ap()[NB2 - 1], has_l1=False)

    nc.compile()
    return nc


_CACHE = {}


def _get_program(T_, S):
    key = (T_, S)
    if key not in _CACHE:
        _CACHE[key] = build_program(T_=T_, S=S)
    return _CACHE[key]


def kernel(x, w_ih1, w_hh1, b_ih1, b_hh1, w_ih2, w_hh2, b_ih2, b_hh2, w_out, b_out,
           _T=None, _S=32):
    x = np.asarray(x, dtype=np.float32)
    T_ = x.shape[1] if _T is None else _T
    nc = _get_program(T_, _S)

    shared = {
        "w1hh": _prep_whh(np.asarray(w_hh1)),
        "w1ih": _prep_wih1(np.asarray(w_ih1), np.asarray(b_ih1) + np.asarray(b_hh1)),
        "w2hh": _prep_whh(np.asarray(w_hh2)),
        "w2ih": _prep_whh(np.asarray(w_ih2)),
        "b2": _prep_b2(np.asarray(b_ih2) + np.asarray(b_hh2)),
        "wout": _prep_wout(np.asarray(w_out)),
        "bout": _prep_bout(np.asarray(b_out)),
    }
    in_maps = []
    for c in range(NC):
        xp_, xe_, xo_ = _prep_x(x[c * BC:(c + 1) * BC], _S)
        in_maps.append({**shared, "xp": xp_, "xe": xe_, "xo": xo_})

    res = bass_utils.run_bass_kernel_spmd(nc, in_maps, core_ids=list(range(NC)))
    y = np.empty((B, T_, D_OUT), np.float32)
    for c in range(NC):
        y[c * BC:(c + 1) * BC] = _reassemble_y(
            res.results[c]["ye"], res.results[c]["yo"], _S, BC, T_)
    return y
